# revision 5
# baseline (speedup 1.0000x reference)
"""TRN2 Bass kernel for the ESN (echo-state-network) recurrence:

    U   = inputs @ W_in + b_in                              # [B, T, N]
    x0  = 0.5 * tanh(U[:, 0])
    x_t = 0.5*x_{t-1} + 0.5*tanh(U[:, t] + x_{t-1} @ W_res + b_res)
    X   = stack([x0 ... x_{T-1}], 1)                        # [B, T, N]

Sharding: TIME-parallel over the sequence with fading-memory warmup.
The ESN map is strongly contracting (leak 0.5, spectral radius 0.9), so
a chunk's initial state can be reconstructed by running L warmup steps
from x=0: the init error decays below the fp16 quantization floor by
L=16 (measured: rel err 4.8e-4 vs 4.8e-4 for exact-init fp16).

Each of the 8 cores runs the FULL batch B=128 for S = 64+L steps
covering output span [64c, 64c+64); cores c>=1 start L steps early
from x=0.  Same program on every core (SPMD); only the input slice
differs.  Core 0 starts at t=0, where the program's step-0 formula
x0 = 0.5*tanh(inp@W_in + b_in) is exactly the reference's first step.

Per-core per-step design ("x-stationary" matmul orientation):
  - z[b, n] accumulates in PSUM [128, 1024] as lhsT.T @ rhs with the
    *state* as stationary (xT tiles [128k, 128b], 8 small loads) and
    W_res as the 128-wide moving operand ([128k, 512n] x 16).  This
    flips the baseline orientation: weight-load time drops from
    64x128 rows/step to 8x128 rows/step, and the moving operand is
    128 wide instead of 16.
  - The input projection + bias is folded in as a K=65 contraction
    chunk (64 input dims + ones row), accumulated into the same PSUM.
    Its two matmuls are issued *before* the previous step's transposes
    so the PE covers the tanh/blend latency tail.
  - tanh on ScalarE (PSUM -> fp16 SBUF), leak blend on VectorE,
    then 8 PE transposes turn x_t [B, N] into next step's stationary
    xT tiles; PSUM->SBUF copies alternate Scalar/Vector.
  - Output is written per step as [B, s, N] fp16; the host slices the
    valid 64-step window per core and concatenates along t.
"""

import sys

sys.path.insert(0, "/opt/trn_rl_repo")

from contextlib import ExitStack

import numpy as np

try:  # persistent jit cache so repeated runs skip long compiles
    import jax

    jax.config.update("jax_compilation_cache_dir", "/var/tmp/jax_comp_cache")
    jax.config.update("jax_persistent_cache_min_compile_time_secs", 0.0)
    jax.config.update("jax_persistent_cache_min_entry_size_bytes", 0)
except Exception:
    pass

import concourse.bass as bass
import concourse.tile as tile
from concourse import bacc, mybir
from concourse.bass_utils import run_bass_kernel_spmd
from concourse.masks import make_identity

F32 = mybir.dt.float32
F16 = mybir.dt.float16

N_CORES = 8
B = 128  # full batch on every core
T = 512
D = 64
N = 1024
NC = 8  # N chunks of 128
P = 128
HN = 512  # half of N (one PSUM bank of fp32)
L = 16  # warmup steps (fading-memory reconstruction)
C = T // N_CORES  # output steps per core
S = C + L  # program steps per core
TANH = mybir.ActivationFunctionType.Tanh
ALU = mybir.AluOpType


def build_kernel():
    nc = bacc.Bacc(None, target_bir_lowering=False)
    # host-side pre-transposed input slice: inputs_t[d, b, s] = inputs[b, t0+s, d]
    inputs = nc.dram_tensor("inputs_t", [D, B, S], F32, kind="ExternalInput")
    W_in = nc.dram_tensor("W_in", [D, N], F32, kind="ExternalInput")
    b_in = nc.dram_tensor("b_in", [N], F32, kind="ExternalInput")
    W_res = nc.dram_tensor("W_res", [N, N], F32, kind="ExternalInput")
    b_res = nc.dram_tensor("b_res", [N], F32, kind="ExternalInput")
    Xs = nc.dram_tensor("Xs", [B, S, N], F16, kind="ExternalOutput")

    K = D + 1  # input dims + ones row (bias via wi row 64)

    with tile.TileContext(nc) as tc, ExitStack() as ctx:
        consts = ctx.enter_context(tc.tile_pool(name="consts", bufs=1))
        stage = ctx.enter_context(tc.tile_pool(name="stage", bufs=1))
        state = ctx.enter_context(tc.tile_pool(name="state", bufs=3))
        zpool = ctx.enter_context(
            tc.tile_pool(name="zpool", bufs=3, space=bass.MemorySpace.PSUM)
        )
        trpool = ctx.enter_context(
            tc.tile_pool(name="trpool", bufs=2, space=bass.MemorySpace.PSUM)
        )

        # ---- identity for PE transposes ----
        ident = consts.tile([P, P], F16, tag="ident")
        make_identity(nc, ident)

        # ---- W_res: wres[p, k, n] = W_res[k*128+p, n] (fp16) ----
        wst = stage.tile([P, NC * N], F32, tag="stage", name="wst")
        nc.gpsimd.dma_start(
            out=wst.rearrange("p (k n) -> p k n", n=N),
            in_=W_res[:].rearrange("(k p) n -> p k n", p=P),
        )
        wres = consts.tile([P, NC, N], F16, tag="wres")
        nc.vector.tensor_copy(out=wres.rearrange("p k n -> p (k n)"), in_=wst)

        # ---- wi tiles [65, N]: rows 0..63 = W_in; row 64 = bias ----
        wi32 = consts.tile([K, N], F32, tag="wi32")
        wi032 = consts.tile([K, N], F32, tag="wi032")
        nc.gpsimd.dma_start(out=wi32[0:D], in_=W_in[:])
        nc.gpsimd.dma_start(out=wi032[0:D], in_=W_in[:])
        nc.gpsimd.dma_start(
            out=wi032[D : D + 1], in_=b_in[:].rearrange("(z n) -> z n", z=1)
        )
        bres_row = consts.tile([K, N], F32, tag="bres")
        nc.gpsimd.dma_start(
            out=bres_row[D : D + 1], in_=b_res[:].rearrange("(z n) -> z n", z=1)
        )
        nc.vector.tensor_tensor(
            out=wi32[D : D + 1],
            in0=wi032[D : D + 1],
            in1=bres_row[D : D + 1],
            op=ALU.add,
        )
        wi = consts.tile([K, N], F16, tag="wi")
        wi0 = consts.tile([K, N], F16, tag="wi0")
        nc.vector.tensor_copy(out=wi, in_=wi32)
        nc.vector.tensor_copy(out=wi0, in_=wi032)

        # ---- inputs: inp[d, b, s] fp16, row 64 = ones ----
        ist = stage.tile([D, B * S], F32, tag="ist")
        nc.sync.dma_start(out=ist, in_=inputs[:].rearrange("d b s -> d (b s)"))
        inp = consts.tile([K, B, S], F16, tag="inp")
        nc.vector.tensor_copy(
            out=inp[0:D].rearrange("d b s -> d (b s)"), in_=ist
        )
        nc.vector.memset(inp[D : D + 1].rearrange("d b s -> d (b s)"), 1.0)

        xs_view = Xs[:]  # [B, S, N]

        # z(s) PSUM tiles are opened 2 steps ahead by their input-projection
        # matmuls (cheap PE filler that covers the tanh/blend/transpose
        # latency tail of the current step); W_res matmuls land later.
        zt = {}

        def inp_mm(s):
            z = zpool.tile([P, N], F32, tag="z", name=f"z{s}")
            zt[s] = z
            wi_use = wi0 if s == 0 else wi
            for h in range(2):
                nc.tensor.matmul(
                    z[:, h * HN : (h + 1) * HN],
                    inp[:, :, s],
                    wi_use[:, h * HN : (h + 1) * HN],
                    start=True,
                    stop=(s == 0),
                    skip_group_check=True,
                )

        xn_prev = None  # x_{s-1} fp16 [128, N] (B-major)
        xh_prev = None  # 0.5 * x_{s-1} fp16 [128, N]
        xT_prev = None  # transposed state tiles [128k, NC, 128b]

        inp_mm(0)
        inp_mm(1)
        inp_mm(2)
        for s in range(S):
            z = zt.pop(s)
            if s > 0:
                # -- recurrent matmuls: z += x_{s-1} @ W_res, half-blocked so
                #    half 0 completes (and its tanh/blend starts) while the
                #    PE streams half 1 --
                for h in range(2):
                    for k in range(NC):
                        nc.tensor.matmul(
                            z[:, h * HN : (h + 1) * HN],
                            xT_prev[:, k, :],
                            wres[:, k, h * HN : (h + 1) * HN],
                            start=False,
                            stop=(k == NC - 1),
                            skip_group_check=True,
                        )
            # -- tanh + leak blend (per half so post-ops pipeline) --
            th = state.tile([P, N], F16, tag="th", name=f"th{s}")
            xn = state.tile([P, N], F16, tag="xn", name=f"xn{s}")
            for h in range(2):
                hs = slice(h * HN, (h + 1) * HN)
                nc.scalar.activation(out=th[:, hs], in_=z[:, hs], func=TANH)
                if s == 0:
                    nc.vector.tensor_scalar_mul(xn[:, hs], th[:, hs], 0.5)
                else:
                    nc.vector.scalar_tensor_tensor(
                        out=xn[:, hs],
                        in0=th[:, hs],
                        scalar=0.5,
                        in1=xh_prev[:, hs],
                        op0=ALU.mult,
                        op1=ALU.add,
                    )
            last = s == S - 1
            if not last:
                # -- transposes of x_s: [B, N] -> 8 xT tiles [128k, 128b].
                #    First half's tiles go right after its blend; the
                #    s+2 input projection fills the PE while half 1's
                #    tanh/blend finishes. --
                trp = trpool.tile([P, NC, P], F16, tag="trp", name=f"trp{s}")
                xT = state.tile([P, NC, P], F16, tag="xT", name=f"xT{s}")
                for k in range(4):
                    nc.tensor.transpose(
                        trp[:, k, :], xn[:, k * P : (k + 1) * P], ident
                    )
                nc.scalar.copy(out=xT[:, 0, :], in_=trp[:, 0, :])
                nc.vector.tensor_copy(out=xT[:, 1, :], in_=trp[:, 1, :])
                nc.scalar.copy(out=xT[:, 2, :], in_=trp[:, 2, :])
                nc.vector.tensor_copy(out=xT[:, 3, :], in_=trp[:, 3, :])
                if s + 2 < S:
                    inp_mm(s + 2)
                for k in range(4, NC):
                    nc.tensor.transpose(
                        trp[:, k, :], xn[:, k * P : (k + 1) * P], ident
                    )
                nc.scalar.copy(out=xT[:, 4, :], in_=trp[:, 4, :])
                nc.vector.tensor_copy(out=xT[:, 5, :], in_=trp[:, 5, :])
                nc.scalar.copy(out=xT[:, 6, :], in_=trp[:, 6, :])
                nc.vector.tensor_copy(out=xT[:, 7, :], in_=trp[:, 7, :])
                # xh = x/2 for the next blend: issued after the copies so the
                # vector queue prioritizes what the PE is waiting on
                xh = state.tile([P, N], F16, tag="xh", name=f"xh{s}")
                for h in range(2):
                    hs = slice(h * HN, (h + 1) * HN)
                    nc.vector.tensor_scalar_mul(xh[:, hs], xn[:, hs], 0.5)
            else:
                xT = xh = None
            nc.sync.dma_start(out=xs_view[:, s, :], in_=xn)
            xn_prev, xh_prev, xT_prev = xn, xh, xT

    nc.compile()
    return nc


_NC_CACHE = {}


def _get_nc():
    if "nc" not in _NC_CACHE:
        _NC_CACHE["nc"] = build_kernel()
    return _NC_CACHE["nc"]


def run_sharded(inputs, W_in, b_in, W_res, b_res, trace=False):
    """Run the SPMD kernel on 8 cores; returns (X_full, BassKernelResults)."""
    b_total, t_steps, _ = inputs.shape
    assert b_total == B and t_steps == T
    nc = _get_nc()
    shared = {
        "W_in": np.ascontiguousarray(W_in, np.float32),
        "b_in": np.ascontiguousarray(b_in, np.float32),
        "W_res": np.ascontiguousarray(W_res, np.float32),
        "b_res": np.ascontiguousarray(b_res, np.float32),
    }
    starts = [0 if c == 0 else C * c - L for c in range(N_CORES)]
    in_maps = [
        {
            "inputs_t": np.ascontiguousarray(
                np.asarray(inputs[:, t0 : t0 + S, :], np.float32).transpose(2, 0, 1)
            ),
            **shared,
        }
        for t0 in starts
    ]
    res = run_bass_kernel_spmd(nc, in_maps, core_ids=list(range(N_CORES)), trace=trace)
    X = np.empty((B, T, N), np.float32)
    for c, r in enumerate(res.results):
        lo = 0 if c == 0 else L
        X[:, C * c : C * (c + 1), :] = r["Xs"][:, lo : lo + C, :].astype(np.float32)
    return X, res


def kernel(**inputs):
    X, _ = run_sharded(
        inputs["inputs"],
        inputs["W_in"],
        inputs["b_in"],
        inputs["W_res"],
        inputs["b_res"],
    )
    return X


# revision 6
# speedup vs baseline: 1.1974x; 1.1974x over previous
"""TRN2 Bass kernel for the ESN (echo-state-network) recurrence:

    U   = inputs @ W_in + b_in                              # [B, T, N]
    x0  = 0.5 * tanh(U[:, 0])
    x_t = 0.5*x_{t-1} + 0.5*tanh(U[:, t] + x_{t-1} @ W_res + b_res)
    X   = stack([x0 ... x_{T-1}], 1)                        # [B, T, N]

Sharding: TIME-parallel over the sequence with fading-memory warmup.
The ESN map is strongly contracting (leak 0.5, spectral radius 0.9), so
a chunk's initial state can be reconstructed by running L warmup steps
from x=0: the init error decays below the fp16 quantization floor by
L=16 (measured: rel err 4.8e-4 vs 4.8e-4 for exact-init fp16).

Each of the 8 cores runs the FULL batch B=128 for S = 64+L steps
covering output span [64c, 64c+64); cores c>=1 start L steps early
from x=0.  Same program on every core (SPMD); only the input slice
differs.  Core 0 starts at t=0, where the program's step-0 formula
x0 = 0.5*tanh(inp@W_in + b_in) is exactly the reference's first step.

Per-core per-step design ("x-stationary" matmul orientation):
  - z[b, n] accumulates in PSUM [128, 1024] as lhsT.T @ rhs with the
    *state* as stationary (xT tiles [128k, 128b], 8 small loads) and
    W_res as the 128-wide moving operand ([128k, 512n] x 16).  This
    flips the baseline orientation: weight-load time drops from
    64x128 rows/step to 8x128 rows/step, and the moving operand is
    128 wide instead of 16.
  - The input projection + bias is folded in as a K=65 contraction
    chunk (64 input dims + ones row), accumulated into the same PSUM.
    Its two matmuls are issued *before* the previous step's transposes
    so the PE covers the tanh/blend latency tail.
  - tanh on ScalarE (PSUM -> fp16 SBUF), leak blend on VectorE,
    then 8 PE transposes turn x_t [B, N] into next step's stationary
    xT tiles; PSUM->SBUF copies alternate Scalar/Vector.
  - Output is written per step as [B, s, N] fp16; the host slices the
    valid 64-step window per core and concatenates along t.
"""

import sys

sys.path.insert(0, "/opt/trn_rl_repo")

from contextlib import ExitStack

import numpy as np

try:  # persistent jit cache so repeated runs skip long compiles
    import jax

    jax.config.update("jax_compilation_cache_dir", "/var/tmp/jax_comp_cache")
    jax.config.update("jax_persistent_cache_min_compile_time_secs", 0.0)
    jax.config.update("jax_persistent_cache_min_entry_size_bytes", 0)
except Exception:
    pass

import concourse.bass as bass
import concourse.tile as tile
from concourse import bacc, mybir
from concourse.bass_utils import run_bass_kernel_spmd
from concourse.masks import make_identity

F32 = mybir.dt.float32
F16 = mybir.dt.float16

N_CORES = 8
B = 128  # full batch on every core
T = 512
D = 64
N = 1024
NC = 8  # N chunks of 128
P = 128
HN = 512  # half of N (one PSUM bank of fp32)
L = 16  # warmup steps (fading-memory reconstruction)
C = T // N_CORES  # output steps per core
S = C + L  # program steps per core
TANH = mybir.ActivationFunctionType.Tanh
ALU = mybir.AluOpType


def build_kernel():
    nc = bacc.Bacc(None, target_bir_lowering=False)
    # host-side pre-transposed input slice: inputs_t[d, b, s] = inputs[b, t0+s, d]
    inputs = nc.dram_tensor("inputs_t", [D, B, S], F32, kind="ExternalInput")
    W_in = nc.dram_tensor("W_in", [D, N], F32, kind="ExternalInput")
    b_in = nc.dram_tensor("b_in", [N], F32, kind="ExternalInput")
    W_res = nc.dram_tensor("W_res", [N, N], F32, kind="ExternalInput")
    b_res = nc.dram_tensor("b_res", [N], F32, kind="ExternalInput")
    Xs = nc.dram_tensor("Xs", [B, S, N], F16, kind="ExternalOutput")

    K = D + 1  # input dims + ones row (bias via wi row 64)

    with tile.TileContext(nc) as tc, ExitStack() as ctx:
        consts = ctx.enter_context(tc.tile_pool(name="consts", bufs=1))
        stage = ctx.enter_context(tc.tile_pool(name="stage", bufs=1))
        state = ctx.enter_context(tc.tile_pool(name="state", bufs=3))
        zpool = ctx.enter_context(
            tc.tile_pool(name="zpool", bufs=3, space=bass.MemorySpace.PSUM)
        )
        trpool = ctx.enter_context(
            tc.tile_pool(name="trpool", bufs=2, space=bass.MemorySpace.PSUM)
        )

        # ---- identity for PE transposes ----
        ident = consts.tile([P, P], F16, tag="ident")
        make_identity(nc, ident)

        # ---- W_res: wres[p, k, n] = W_res[k*128+p, n] (fp16) ----
        wst = stage.tile([P, NC * N], F32, tag="stage", name="wst")
        nc.gpsimd.dma_start(
            out=wst.rearrange("p (k n) -> p k n", n=N),
            in_=W_res[:].rearrange("(k p) n -> p k n", p=P),
        )
        wres = consts.tile([P, NC, N], F16, tag="wres")
        nc.vector.tensor_copy(out=wres.rearrange("p k n -> p (k n)"), in_=wst)

        # ---- wi tiles [65, N]: rows 0..63 = W_in; row 64 = bias ----
        wi32 = consts.tile([K, N], F32, tag="wi32")
        wi032 = consts.tile([K, N], F32, tag="wi032")
        nc.gpsimd.dma_start(out=wi32[0:D], in_=W_in[:])
        nc.gpsimd.dma_start(out=wi032[0:D], in_=W_in[:])
        nc.gpsimd.dma_start(
            out=wi032[D : D + 1], in_=b_in[:].rearrange("(z n) -> z n", z=1)
        )
        bres_row = consts.tile([K, N], F32, tag="bres")
        nc.gpsimd.dma_start(
            out=bres_row[D : D + 1], in_=b_res[:].rearrange("(z n) -> z n", z=1)
        )
        nc.vector.tensor_tensor(
            out=wi32[D : D + 1],
            in0=wi032[D : D + 1],
            in1=bres_row[D : D + 1],
            op=ALU.add,
        )
        wi = consts.tile([K, N], F16, tag="wi")
        wi0 = consts.tile([K, N], F16, tag="wi0")
        nc.vector.tensor_copy(out=wi, in_=wi32)
        nc.vector.tensor_copy(out=wi0, in_=wi032)

        # ---- inputs: inp[d, b, s] fp16, row 64 = ones ----
        ist = stage.tile([D, B * S], F32, tag="ist")
        nc.sync.dma_start(out=ist, in_=inputs[:].rearrange("d b s -> d (b s)"))
        inp = consts.tile([K, B, S], F16, tag="inp")
        nc.vector.tensor_copy(
            out=inp[0:D].rearrange("d b s -> d (b s)"), in_=ist
        )
        nc.vector.memset(inp[D : D + 1].rearrange("d b s -> d (b s)"), 1.0)

        xs_view = Xs[:]  # [B, S, N]

        # Software-pipelined step loop.  Iteration s emits (PE queue):
        #   inp(s+1) | tr(s-1) k4..7 | W(s) h0 k0..7 | W(s) h1 k0..6 |
        #   tr(s) k0..3 | W(s) h1 k7
        # so: the early k-chunk transposes of step s run *inside* step s's
        # half-1 matmul block (their blend is ready by then), the late
        # chunks' chain (tanh h1 -> blend h1 -> tr k4..7 -> copies) overlaps
        # the next iteration's inp matmuls + W h0 k0..3 (which consume the
        # early chunks first), and every LDW's operand is produced nearly a
        # full W-block before its use.
        zt = {}

        def inp_mm(s):
            z = zpool.tile([P, N], F32, tag="z", name=f"z{s}")
            zt[s] = z
            wi_use = wi0 if s == 0 else wi
            for h in range(2):
                nc.tensor.matmul(
                    z[:, h * HN : (h + 1) * HN],
                    inp[:, :, s],
                    wi_use[:, h * HN : (h + 1) * HN],
                    start=True,
                    stop=(s == 0),
                    skip_group_check=True,
                )

        def w_mm(z, xT, h, ks, stop=False):
            for k in ks:
                nc.tensor.matmul(
                    z[:, h * HN : (h + 1) * HN],
                    xT[:, k, :],
                    wres[:, k, h * HN : (h + 1) * HN],
                    start=False,
                    stop=stop and k == ks[-1],
                    skip_group_check=True,
                )

        def tr_mm(trp, xn, ks):
            for k in ks:
                nc.tensor.transpose(trp[:, k, :], xn[:, k * P : (k + 1) * P], ident)

        def copies(xT, trp, ks):
            for j, k in enumerate(ks):
                cp = nc.scalar.copy if j % 2 == 0 else nc.vector.tensor_copy
                cp(out=xT[:, k, :], in_=trp[:, k, :])

        xn_prev = xh_prev = None
        xT_prev = trp_prev = None  # step s-1 state tiles (k4..7 pending)

        inp_mm(0)
        inp_mm(1)
        for s in range(S):
            last = s == S - 1
            z = zt.pop(s)
            if s + 1 < S:
                inp_mm(s + 1)
            if s > 0:
                if xT_prev is not None and trp_prev is not None:
                    # late transposes + copies of step s-1 (need blend h1(s-1))
                    tr_mm(trp_prev, xn_prev, range(4, NC))
                    copies(xT_prev, trp_prev, range(4, NC))
                # recurrent matmuls; h0 completes early so its tanh/blend/
                # transposes overlap the h1 block
                w_mm(z, xT_prev, 0, list(range(NC)), stop=True)
                w_mm(z, xT_prev, 1, list(range(NC - 1)))
            th = state.tile([P, N], F16, tag="th", name=f"th{s}")
            xn = state.tile([P, N], F16, tag="xn", name=f"xn{s}")
            hs0, hs1 = slice(0, HN), slice(HN, N)
            nc.scalar.activation(out=th[:, hs0], in_=z[:, hs0], func=TANH)
            if s == 0:
                nc.vector.tensor_scalar_mul(xn[:, hs0], th[:, hs0], 0.5)
            else:
                nc.vector.scalar_tensor_tensor(
                    out=xn[:, hs0], in0=th[:, hs0], scalar=0.5,
                    in1=xh_prev[:, hs0], op0=ALU.mult, op1=ALU.add,
                )
            if not last:
                trp = trpool.tile([P, NC, P], F16, tag="trp", name=f"trp{s}")
                xT = state.tile([P, NC, P], F16, tag="xT", name=f"xT{s}")
                tr_mm(trp, xn, range(4))  # early transposes (inside h1 block)
                copies(xT, trp, range(4))
            else:
                trp = xT = None
            if s > 0:
                w_mm(z, xT_prev, 1, [NC - 1], stop=True)
            nc.scalar.activation(out=th[:, hs1], in_=z[:, hs1], func=TANH)
            if s == 0:
                nc.vector.tensor_scalar_mul(xn[:, hs1], th[:, hs1], 0.5)
            else:
                nc.vector.scalar_tensor_tensor(
                    out=xn[:, hs1], in0=th[:, hs1], scalar=0.5,
                    in1=xh_prev[:, hs1], op0=ALU.mult, op1=ALU.add,
                )
            if not last:
                xh = state.tile([P, N], F16, tag="xh", name=f"xh{s}")
                nc.vector.tensor_scalar_mul(xh[:, hs0], xn[:, hs0], 0.5)
                nc.vector.tensor_scalar_mul(xh[:, hs1], xn[:, hs1], 0.5)
            else:
                xh = None
            nc.sync.dma_start(out=xs_view[:, s, :], in_=xn)
            xn_prev, xh_prev, xT_prev, trp_prev = xn, xh, xT, trp

    nc.compile()
    return nc


_NC_CACHE = {}


def _get_nc():
    if "nc" not in _NC_CACHE:
        _NC_CACHE["nc"] = build_kernel()
    return _NC_CACHE["nc"]


def run_sharded(inputs, W_in, b_in, W_res, b_res, trace=False):
    """Run the SPMD kernel on 8 cores; returns (X_full, BassKernelResults)."""
    b_total, t_steps, _ = inputs.shape
    assert b_total == B and t_steps == T
    nc = _get_nc()
    shared = {
        "W_in": np.ascontiguousarray(W_in, np.float32),
        "b_in": np.ascontiguousarray(b_in, np.float32),
        "W_res": np.ascontiguousarray(W_res, np.float32),
        "b_res": np.ascontiguousarray(b_res, np.float32),
    }
    starts = [0 if c == 0 else C * c - L for c in range(N_CORES)]
    in_maps = [
        {
            "inputs_t": np.ascontiguousarray(
                np.asarray(inputs[:, t0 : t0 + S, :], np.float32).transpose(2, 0, 1)
            ),
            **shared,
        }
        for t0 in starts
    ]
    res = run_bass_kernel_spmd(nc, in_maps, core_ids=list(range(N_CORES)), trace=trace)
    X = np.empty((B, T, N), np.float32)
    for c, r in enumerate(res.results):
        lo = 0 if c == 0 else L
        X[:, C * c : C * (c + 1), :] = r["Xs"][:, lo : lo + C, :].astype(np.float32)
    return X, res


def kernel(**inputs):
    X, _ = run_sharded(
        inputs["inputs"],
        inputs["W_in"],
        inputs["b_in"],
        inputs["W_res"],
        inputs["b_res"],
    )
    return X
